# revision 43
# baseline (speedup 1.0000x reference)
"""Batched ragged segment-mean (BERTEmbedder merge loop) on 8 TRN2 NeuronCores.

Strategy
--------
Data-parallel over the batch: each of the 8 cores processes 2 of the 16
sequences (assignment chosen by the host, see below).  Within a sequence,
segment-sum is computed as a block-sparse one-hot matmul on the PE:

    out[t, d] = sum_s onehot[s, t] * x[s, d]

Segment ids are sorted per row, so each 128-subtoken tile only covers a
narrow window of token ids.  The host inspects the ids and builds a static
(s_tile, t_tile) pair schedule: for each s-tile we emit matmuls only
into the 128-row t-tiles its ids can touch (union over the sequences that
share the SPMD program slot, so one program serves all 8 cores).  The host
also precomputes the per-token reciprocal counts (it already scans the ids
for the schedule), so the device only multiplies sums by 1/count — no
count column, no on-device reciprocal, and the x loads stay fully
contiguous per partition.

The kernel is DMA-bound (37.75 MB of HBM traffic per core; measured
combined read+write ceiling ~410 GB/s).  A single 1-cycle-per-row fp32r
matmul per (s-tile, t-tile) pair suffices: the harness gate is 2e-2
relative and one rounded pass (~11 mantissa bits) lands at ~2e-4.

Engine dataflow is strictly separated so no queue ever head-of-line
blocks another: SYNC ring = x loads only; DVE = pre-matmul work only
(one-hots + fp32r rounding copies, free-running ahead of the PE); ACT =
post-matmul work only (both finalize halves, then the store DMA on the
ACT HWDGE ring right behind them, so its issue wait is ~zero); GpSimd =
the tiny sid/recip loads.  Each t-tile stores individually the moment it
closes, spreading write bandwidth across the stream.  Groups are
[4,...,4,2,1,1]: full-size groups keep the HBM read stream coarse, the
tiny last groups shorten the load->matmul->finalize->store drain chain.
The 16 sequences are assigned to the two SPMD program slots by searching
all 6435 8/8 partitions for the one minimizing total union-schedule pairs.
"""

import os
import numpy as np

B, S, D, T, P = 16, 4096, 768, 2048, 128
NCORES = 8
SPC = B // NCORES          # sequences per core
NST, NTT = S // P, T // P  # 32 s-tiles, 16 t-tiles
DSPLIT = 512               # PSUM bank limit (fp32 words)
GROUPS = [4, 4, 4, 4, 4, 4, 4, 1, 1, 1, 1]  # s-tiles per x-load DMA (= NST)
QUAD = 1                   # t-tiles per output store

_cache: dict = {}


def _schedule(segment_ids: np.ndarray):
    """Per program slot q: which t-tiles each s-tile touches, unioned over the
    sequences that run in that slot on every core (SPMD: one program)."""
    from itertools import combinations
    mins = segment_ids.reshape(B, NST, P).min(2) // P
    maxs = segment_ids.reshape(B, NST, P).max(2) // P

    def _npairs(group):
        return int((maxs[list(group)].max(0) - mins[list(group)].min(0) + 1).sum())

    best = None
    allseq = set(range(B))
    for combo in combinations(range(1, B), NCORES - 1):
        g0 = (0,) + combo
        g1 = tuple(sorted(allseq - set(g0)))
        c = _npairs(g0) + _npairs(g1)
        if best is None or c < best[0]:
            best = (c, (g0, g1))
    slot_seqs = best[1]

    sched = []
    for q in range(SPC):
        seqs = list(slot_seqs[q])
        js_of = []
        for i in range(NST):
            blk = segment_ids[seqs, i * P:(i + 1) * P]
            lo, hi = int(blk.min()), int(blk.max())
            js_of.append(list(range(lo // P, hi // P + 1)))
        first, last = {}, {}
        for i in range(NST):
            for j in js_of[i]:
                first.setdefault(j, i)
                last[j] = i
        # loud guard: the PSUM accumulator pools have 4 slots each; more
        # simultaneously-open t-tiles would deadlock the tile scheduler
        maxopen = max(sum(1 for j in first if first[j] <= i <= last[j])
                      for i in range(NST))
        assert maxopen <= 3, f"schedule needs {maxopen} open PSUM accumulators"
        sched.append((tuple(tuple(js) for js in js_of),
                      tuple(sorted(first.items())),
                      tuple(sorted(last.items()))))
    return tuple(sched), slot_seqs


def _build(sched):
    from contextlib import ExitStack
    import concourse.bacc as bacc
    import concourse.tile as tile
    import concourse.mybir as mybir

    f32, f32r, i32 = mybir.dt.float32, mybir.dt.float32r, mybir.dt.int32
    AO = mybir.AluOpType
    nc = bacc.Bacc("TRN2", target_bir_lowering=False, debug=False)
    x = nc.dram_tensor("raw_output", [SPC, S, D], f32, kind="ExternalInput").ap()
    sid = nc.dram_tensor("segment_ids", [SPC, S], i32, kind="ExternalInput").ap()
    recip = nc.dram_tensor("recip", [SPC, T], f32, kind="ExternalInput").ap()
    out = nc.dram_tensor("out", [SPC, T, D], f32, kind="ExternalOutput").ap()

    GMAX = max(GROUPS)
    goff = [0]
    for glen in GROUPS:
        goff.append(goff[-1] + glen)

    with ExitStack() as ctx:
        tc = ctx.enter_context(tile.TileContext(nc))
        const = ctx.enter_context(tc.tile_pool(name="const", bufs=1))
        xp = ctx.enter_context(tc.tile_pool(name="xp", bufs=6))
        hip = ctx.enter_context(tc.tile_pool(name="hip", bufs=4))
        ohp = ctx.enter_context(tc.tile_pool(name="ohp", bufs=12))
        outp = ctx.enter_context(tc.tile_pool(name="outp", bufs=6))
        smp = ctx.enter_context(tc.tile_pool(name="smp", bufs=4))
        psb = ctx.enter_context(tc.tile_pool(name="psb", bufs=4, space="PSUM"))

        maxw = P * max(len(js) for q in range(SPC) for js in sched[q][0])
        iota_i = const.tile([P, maxw], i32)
        nc.gpsimd.iota(iota_i[:], pattern=[[1, maxw]], base=0, channel_multiplier=0)
        iota_w = const.tile([P, maxw], f32)
        nc.vector.tensor_copy(iota_w[:], iota_i[:])
        iota_f = iota_w[:, 0:P]
        pidx_i = const.tile([P, 1], i32)
        nc.gpsimd.iota(pidx_i[:], pattern=[[1, 1]], base=0, channel_multiplier=1)
        pidx_f = const.tile([P, 1], f32)
        nc.vector.tensor_copy(pidx_f[:], pidx_i[:])
        # identity[p, f] = (iota[f] == p), used by the PE transpose
        ident = const.tile([NST, NST], f32)
        nc.vector.tensor_scalar(ident[:], iota_f[0:NST, 0:NST], pidx_f[0:NST],
                                None, AO.is_equal)
        # dummy activation: triggers the lazy ACT_TABLE_LOAD (~1.3us)
        # during the preamble instead of in front of the first finalize
        warm = const.tile([P, 1], f32)
        nc.scalar.activation(warm[:], pidx_f[:],
                             mybir.ActivationFunctionType.Copy)

        # segment ids for all 32 s-tiles of both slots -> [128, 32] per slot,
        # hoisted to the program start so the PE transpose clears early
        sid_alls, rec_alls = [], []
        for q in range(SPC):
            # sid/recip lead the SYNC ring: tiny transfers, and the SWDGE
            # (gpsimd) alternative delivers them ~4us later, gating the
            # first matmul
            sid32_i = smp.tile([NST, P], i32, tag="sid32i", name=f"sid32i_{q}")
            nc.sync.dma_start(out=sid32_i[:],
                              in_=sid[q].rearrange("(n p) -> n p", p=P))
            # recip loads contiguous as [NTT, P], then rides the same PE
            # transpose as the ids to land token-major [P, NTT]
            rec16 = smp.tile([NTT, P], f32, tag="rec16", name=f"rec16_{q}")
            nc.sync.dma_start(out=rec16[:],
                              in_=recip[q].rearrange("(n p) -> n p", p=P))
            recT_ps = psb.tile([P, NTT], f32, tag="psB", name=f"recT_{q}")
            nc.tensor.transpose(recT_ps[:], rec16[:], ident[0:NTT, 0:NTT])
            rec_all = smp.tile([P, NTT], f32, tag="rec_all", name=f"rec_all_{q}")
            nc.vector.tensor_copy(rec_all[:], recT_ps[:])
            rec_alls.append(rec_all)
            sid32 = smp.tile([NST, P], f32, tag="sid32", name=f"sid32_{q}")
            nc.vector.tensor_copy(sid32[:], sid32_i[:])
            sidT_ps = psb.tile([P, NST], f32, tag="psA", name=f"sidT_{q}")
            nc.tensor.transpose(sidT_ps[:], sid32[:], ident[:])
            sid_all = smp.tile([P, NST], f32, tag="sid_all", name=f"sid_all_{q}")
            nc.vector.tensor_copy(sid_all[:], sidT_ps[:])
            sid_alls.append(sid_all)

        ctxs = []
        for q in range(SPC):
            js_of, first_t, last_t = sched[q]
            ctxs.append({
                "js_of": js_of, "first": dict(first_t), "last": dict(last_t),
                "sid_all": sid_alls[q], "rec_all": rec_alls[q],
                "x_seq": x[q].rearrange("(n p) d -> p n d", p=P),
                "out_seq": out[q].rearrange("(n p) d -> p n d", p=P),
                "open_ps": {}, "pend_out": {}})

        def emit_group(q, g):
            c = ctxs[q]
            js_of, first, last = c["js_of"], c["first"], c["last"]
            sid_all, open_ps, pend_out = c["sid_all"], c["open_ps"], c["pend_out"]
            lo_i, glen = goff[g], GROUPS[g]
            xt = xp.tile([P, glen, D], f32, tag="xt", name=f"xt_q{q}_g{g}")
            nc.sync.dma_start(out=xt[:],
                              in_=c["x_seq"][:, lo_i:lo_i + glen, :])
            # one-hot windows first: they only depend on sid_all, so the
            # DVE can produce them while the x DMA is still in flight
            ohws = []
            for si in range(glen):
                i = lo_i + si
                js = js_of[i]
                ohw = ohp.tile([P, P * len(js)], f32r, tag="oh",
                               name=f"oh_q{q}_i{i}")
                nc.vector.tensor_scalar(
                    ohw[:], iota_w[:, 0:P * len(js)], float(js[0] * P),
                    sid_all[:, i:i + 1], AO.add, AO.is_equal)
                ohws.append(ohw)
            # single rounded fp32r pass (the BIR verifier requires an
            # explicit rounding producer).  All rounding lives on the DVE:
            # the DVE then has no post-matmul dependencies at all, so it
            # free-runs ahead of the PE (bounded only by hip bufs)
            xr = hip.tile([P, glen, D], f32r, tag="hi", name=f"hi_q{q}_g{g}")
            for h in range(0, glen, 2):
                h2 = min(h + 2, glen)
                nc.vector.tensor_copy(xr[:, h:h2, :], xt[:, h:h2, :])
            for si in range(glen):
                i = lo_i + si
                for k, j in enumerate(js_of[i]):
                    st = first[j] == i
                    sp_ = last[j] == i
                    if st:
                        open_ps[j] = (
                            psb.tile([P, DSPLIT], f32, tag="psA",
                                     name=f"accA_q{q}_j{j}"),
                            psb.tile([P, D - DSPLIT], f32, tag="psB",
                                     name=f"accB_q{q}_j{j}"))
                    pa, pb = open_ps[j]
                    oh = ohws[si][:, k * P:(k + 1) * P]
                    nc.tensor.matmul(pa[:], lhsT=oh, rhs=xr[:, si, 0:DSPLIT],
                                     start=st, stop=sp_)
                    nc.tensor.matmul(pb[:], lhsT=oh, rhs=xr[:, si, DSPLIT:D],
                                     start=st, stop=sp_)
                    if sp_:
                        rec = c["rec_all"][:, j:j + 1]
                        jp = j // QUAD
                        if jp not in pend_out:
                            pend_out[jp] = [outp.tile([P, QUAD, D], f32,
                                                      tag="ot",
                                                      name=f"ot_q{q}_{jp}"), 0]
                        ot, _ = pend_out[jp]
                        sl = j % QUAD
                        # both finalize halves on ACT: the store right after
                        # then waits only on ACT work emitted immediately
                        # before it — no cross-engine store stalls.
                        # Exception: t-tiles closing in the drain (last 4
                        # s-tiles), where several close at once and ACT
                        # serializes — there the DVE (idle by then, no
                        # rounding left to head-of-line block) takes the
                        # second half so the two run in parallel
                        nc.scalar.activation(ot[:, sl, 0:DSPLIT], pa[:],
                                             mybir.ActivationFunctionType.Copy,
                                             scale=rec)
                        if last[j] >= NST - 4:
                            nc.vector.tensor_scalar_mul(
                                ot[:, sl, DSPLIT:D], pb[:], rec)
                        else:
                            nc.scalar.activation(
                                ot[:, sl, DSPLIT:D], pb[:],
                                mybir.ActivationFunctionType.Copy, scale=rec)
                        pend_out[jp][1] += 1
                        if pend_out[jp][1] == QUAD:
                            # the store rides the ACT HWDGE ring directly
                            # behind its own finalize: its issue wait is
                            # always ~zero, so it can never head-of-line
                            # block anything
                            nc.scalar.dma_start(
                                out=c["out_seq"][:, QUAD * jp:QUAD * (jp + 1), :],
                                in_=ot[:])
                            del pend_out[jp]
                        del open_ps[j]

        # interleave the two slots' groups: two independent dependency
        # chains keep every engine fed through the other chain's stalls
        for g in range(len(GROUPS)):
            for q in range(SPC):
                emit_group(q, g)

        for q in range(SPC):
            c = ctxs[q]
            first, pend_out, out_seq = c["first"], c["pend_out"], c["out_seq"]
            # flush quads whose remaining t-tiles can never finalize
            for jp, (ot, n) in list(pend_out.items()):
                for sl in range(QUAD):
                    if QUAD * jp + sl not in first:
                        nc.vector.memset(ot[:, sl, :], 0.0)
                nc.scalar.dma_start(out=out_seq[:, QUAD * jp:QUAD * (jp + 1), :],
                                    in_=ot[:])
            # quads no s-tile can touch at all
            for jp in range(NTT // QUAD):
                if jp not in pend_out and \
                        all(QUAD * jp + sl not in first for sl in range(QUAD)):
                    zt = outp.tile([P, QUAD, D], f32, tag="ot",
                                   name=f"zt_q{q}_{jp}")
                    nc.vector.memset(zt[:], 0.0)
                    nc.scalar.dma_start(out=out_seq[:, QUAD * jp:QUAD * (jp + 1), :],
                                        in_=zt[:])
    nc.compile()
    return nc


def _get_nc(segment_ids: np.ndarray):
    sched, slot_seqs = _schedule(segment_ids)
    if sched not in _cache:
        _cache[sched] = _build(sched)
    return _cache[sched], slot_seqs


def run(raw_output, segment_ids, trace=False):
    from concourse.bass_utils import run_bass_kernel_spmd

    raw_output = np.ascontiguousarray(raw_output, dtype=np.float32)
    segment_ids = np.ascontiguousarray(segment_ids, dtype=np.int32)
    nc, slot_seqs = _get_nc(segment_ids)
    cnts = np.zeros((B, T), np.float32)
    for b in range(B):
        cnts[b] = np.bincount(segment_ids[b], minlength=T)
    recip = (1.0 / np.maximum(cnts, 1.0)).astype(np.float32)
    in_maps = []
    for c in range(NCORES):
        seqs = [slot_seqs[q][c] for q in range(SPC)]
        in_maps.append({
            "raw_output": np.ascontiguousarray(raw_output[seqs]),
            "segment_ids": np.ascontiguousarray(segment_ids[seqs]),
            "recip": np.ascontiguousarray(recip[seqs])})
    bkr = run_bass_kernel_spmd(nc, in_maps, list(range(NCORES)), trace=trace)
    full = np.empty((B, T, D), np.float32)
    for c in range(NCORES):
        for q in range(SPC):
            full[slot_seqs[q][c]] = bkr.results[c]["out"][q]
    return full, bkr


def kernel(raw_output, segment_ids):
    full, _ = run(raw_output, segment_ids,
                  trace=bool(int(os.environ.get("KERNEL_TRACE", "0"))))
    return full
